# revision 1
# baseline (speedup 1.0000x reference)
"""NWNet (retrieval-knn) Trainium2 kernel, 8 NeuronCores — all-SWI rep, deep-pipelined.

Every matmul in the rep body runs in fp8 perf_mode=DoubleRowSwInterleave
(SWI), so the PE's weight path never switches modes (measured ~3.5us per
SWI<->other transition). SWI reverses the stationary's logical column
order; the reversal applied once in phase 3 (queries -> PSUM partitions)
and once in phase 4 (probs -> output partitions) cancels, so the final
output is natural-order. Support norms come from a full-width all-ones SWI
matmul (whose 128-partition output replicates the norm row — a free
partition broadcast) and are added by a fused DVE affine_then_add;
query norms enter via the sqrt activation's per-partition f32 bias
(produced reversed by an SWI ones-matmul over the funky-layout squares).
"""

import numpy as np
import ml_dtypes

import concourse.bacc as bacc
import concourse.mybir as mybir
import concourse.tile as tile
from concourse.bass_utils import run_bass_kernel_spmd

FP8 = mybir.dt.float8e4
BF16 = mybir.dt.bfloat16
F32 = mybir.dt.float32

B = 128
S_C = 1024
FIN = 4096
PD = 1024
CPAD = 1024
N = B + S_C
KC = FIN // 128
KC2 = KC // 2
PC = PD // 128
PC2 = PC // 2    # 4 chunk pairs
SC = S_C // 128
CC = CPAD // 128

SCALE = 32.0
K_OFF = 50.0
EPS = 1e-12


def build_bass(reps=1):
    nc = bacc.Bacc("TRN2", target_bir_lowering=False, debug=False, num_devices=8)

    wp_d = nc.dram_tensor("wp", [128, PC, KC * 128], FP8, kind="ExternalInput")
    rxt_d = nc.dram_tensor("rxt", [128, KC, N], FP8, kind="ExternalInput")
    oh_d = nc.dram_tensor("oh", [128, SC, CPAD], FP8, kind="ExternalInput")
    out_d = nc.dram_tensor("outp", [B, CPAD], F32, kind="ExternalOutput")

    Act = mybir.ActivationFunctionType
    SWI = mybir.MatmulPerfMode.DoubleRowSwInterleave

    with tile.TileContext(nc) as tc:
        with (
            tc.tile_pool(name="rxt", bufs=1) as p_rxt,
            tc.tile_pool(name="w", bufs=1) as p_w,
            tc.tile_pool(name="qs", bufs=2) as p_qs,
            tc.tile_pool(name="oh", bufs=1) as p_oh,
            tc.tile_pool(name="sq", bufs=2) as p_sq,
            tc.tile_pool(name="nsq", bufs=1) as p_nsq,
            tc.tile_pool(name="nsq2", bufs=2) as p_nsq2,
            tc.tile_pool(name="dist", bufs=2) as p_dist,
            tc.tile_pool(name="probs", bufs=2) as p_probs,
            tc.tile_pool(name="osb", bufs=2) as p_osb,
            tc.tile_pool(name="ps8", bufs=8, space="PSUM") as p_ps,
        ):
            # ---- resident input loads (once per NEFF) ----
            rxt_sb = p_rxt.tile([128, KC, N], FP8)
            for g in range(8):
                nc.sync.dma_start(
                    out=rxt_sb[:, g * 4 : (g + 1) * 4, :],
                    in_=rxt_d[:, g * 4 : (g + 1) * 4, :],
                )
            wp_sb = p_w.tile([128, PC, KC2, 256], FP8)
            for m2 in range(PC):
                nc.sync.dma_start(out=wp_sb[:, m2], in_=wp_d[:, m2])
            oh_sb = p_oh.tile([128, SC, CPAD], FP8)
            nc.sync.dma_start(out=oh_sb[:], in_=oh_d[:])
            ones2 = p_nsq.tile([128, 2, 16], FP8, tag="ones2")
            nc.vector.memset(ones2[:], 1.0)
            # full-width all-ones SWI stationary: SWI LDWEIGHTS requires 256
            # active columns, and the 128-partition output usefully replicates
            # the norm row across partitions (a free partition-broadcast)
            ones_full = p_nsq.tile([128, 2, 128], FP8, tag="ones_full")
            nc.vector.memset(ones_full[:], 1.0)
            koff_sb = p_nsq.tile([128, 1], F32, tag="koff")
            nc.vector.memset(koff_sb[:], K_OFF)

            def phase1_m2(m2, qs8, qsw, sqall, sqw):
                ps = [
                    p_ps.tile([128, 512], F32, tag="bank", name=f"mmps{h}")
                    for h in range(2)
                ]
                psq = p_ps.tile([128, B], F32, tag="bank", name="mmpsq")
                for kc2 in range(KC2):
                    lhs = wp_sb[:, m2, kc2, :]
                    st, sp = kc2 == 0, kc2 == KC2 - 1
                    for h in range(2):
                        nc.tensor.matmul(
                            ps[h][:],
                            lhs,
                            rxt_sb[
                                :, 2 * kc2 : 2 * kc2 + 2,
                                B + h * 512 : B + (h + 1) * 512,
                            ],
                            start=st,
                            stop=sp,
                            perf_mode=SWI,
                        )
                    nc.tensor.matmul(
                        psq[:],
                        lhs,
                        rxt_sb[:, 2 * kc2 : 2 * kc2 + 2, 0:B],
                        start=st,
                        stop=sp,
                        perf_mode=SWI,
                    )
                a, i = m2 // 2, m2 % 2
                nc.scalar.copy(qs8[:, m2, 0:512], ps[0][:])
                nc.vector.tensor_copy(qs8[:, m2, 512:1024], ps[1][:])
                nc.vector.tensor_copy(qsw[:, a, :, i], psq[:])  # stride-2 funky
                # scale 1/SCALE so the squares sum directly to unscaled norms
                nc.scalar.activation(
                    sqall[:, m2, :], qs8[:, m2, :], Act.Square, bias=0.0,
                    scale=1.0 / SCALE,
                )
                nc.scalar.activation(
                    sqw[:, a, :, i], qsw[:, a, :, i], Act.Square, bias=0.0,
                    scale=1.0 / SCALE,
                )

            def phase4(probs8, out_sb):
                pos = [
                    p_ps.tile([B, 512], F32, tag="bank", name=f"po{h}")
                    for h in range(2)
                ]
                for j in range(4):
                    for h in range(2):
                        nc.tensor.matmul(
                            pos[h][:],
                            probs8[:, j, :, :],
                            oh_sb[:, 2 * j : 2 * j + 2, h * 512 : (h + 1) * 512],
                            start=(j == 0),
                            stop=(j == 3),
                            perf_mode=SWI,
                        )
                for h in range(2):
                    nc.vector.tensor_copy(
                        out_sb[:, h * 512 : (h + 1) * 512], pos[h][:]
                    )
                    nc.sync.dma_start(
                        out=out_d[:, h * 512 : (h + 1) * 512],
                        in_=out_sb[:, h * 512 : (h + 1) * 512],
                    )

            def phase3(qs8, qsw, nsqB, bias_q):
                probs_qs = p_probs.tile([128, S_C], BF16, tag="pqs")
                probs_t = p_probs.tile([128, PC2, 2, 128], BF16, tag="pt")
                probs8 = p_probs.tile([128, PC2, 128, 2], FP8, tag="p8")
                gts = [
                    p_ps.tile([128, 512], F32, tag="bank", name=f"gt{h}")
                    for h in range(2)
                ]
                for a in range(PC2):
                    lhsT = qsw[:, a, :, :]
                    for h in range(2):
                        nc.tensor.matmul(
                            gts[h][:],
                            lhsT,
                            qs8[:, 2 * a : 2 * a + 2, h * 512 : (h + 1) * 512],
                            start=(a == 0),
                            stop=(a == PC2 - 1),
                            perf_mode=SWI,
                        )
                for h in range(2):
                    tmp = p_dist.tile([128, 512], F32, tag="dist")
                    nc.vector.affine_then_add(
                        tmp[:],
                        gts[h][:],
                        nsqB[:, h * 512 : (h + 1) * 512],
                        scale=-2.0 / SCALE**2,
                        bias=0.0,
                    )
                    distq = p_dist.tile([128, 512], F32, tag="dist2")
                    nc.scalar.activation(
                        distq[:], tmp[:], Act.Sqrt, bias=bias_q[:, 0:1], scale=1.0
                    )
                    nc.scalar.activation(
                        probs_qs[:, h * 512 : (h + 1) * 512],
                        distq[:],
                        Act.Exp,
                        bias=koff_sb[:],
                        scale=-1.0,
                    )
                    for j in range(4):
                        sc = h * 4 + j
                        nc.sync.dma_start(
                            out=probs_t[:, sc // 2, sc % 2, :],
                            in_=probs_qs[:, sc * 128 : (sc + 1) * 128],
                            transpose=True,
                        )
                    for i in range(2):
                        nc.vector.tensor_copy(
                            probs8[:, 2 * h : 2 * h + 2, :, i],
                            probs_t[:, 2 * h : 2 * h + 2, i, :],
                        )
                out_sb = p_osb.tile([128, CPAD], F32)
                return (probs8, out_sb)

            pending3 = None
            pending4 = None
            for _rep in range(reps):
                # ---- phase 1, with rep i-1's phases 3/4 pipelined in so the
                # probs chain (ACT + DMA transposes) gets matmul-passes of
                # slack and never stalls the tensor engine ----
                qs8 = p_qs.tile([128, PC, S_C], FP8)
                qsw = p_qs.tile([128, PC2, 128, 2], FP8, tag="qsw")
                sqall = p_sq.tile([128, PC, S_C], FP8)
                sqw = p_sq.tile([128, PC2, 128, 2], FP8, tag="sqw")
                phase1_m2(0, qs8, qsw, sqall, sqw)
                phase1_m2(1, qs8, qsw, sqall, sqw)
                if pending3 is not None:
                    pending4 = phase3(*pending3)
                    pending3 = None
                phase1_m2(2, qs8, qsw, sqall, sqw)
                phase1_m2(3, qs8, qsw, sqall, sqw)
                phase1_m2(4, qs8, qsw, sqall, sqw)
                if pending4 is not None:
                    phase4(*pending4)
                    pending4 = None
                for m2 in range(5, PC):
                    phase1_m2(m2, qs8, qsw, sqall, sqw)

                # ---- phase 2: norms ----
                # support norms along free: nps_s = sum_p sq/256  (x2 halves)
                nps_s = [
                    p_ps.tile([128, 512], F32, tag="bank", name=f"nps{h}")
                    for h in range(2)
                ]
                for j in range(PC2):
                    for h in range(2):
                        nc.tensor.matmul(
                            nps_s[h][:],
                            ones_full[:],
                            sqall[:, 2 * j : 2 * j + 2, h * 512 : (h + 1) * 512],
                            start=(j == 0),
                            stop=(j == PC2 - 1),
                            perf_mode=SWI,
                        )
                # query norms onto partitions (SWI column-reversal gives the
                # reversed order phase 3's PSUM layout needs)
                qn_ps = p_ps.tile([128, 1], F32, tag="bank", name="qnps")
                for j in range(PC2):
                    nc.tensor.matmul(
                        qn_ps[:, 0:1],
                        sqw[:, j, :, :],
                        ones2[:, :, 0:1],
                        start=(j == 0),
                        stop=(j == PC2 - 1),
                        perf_mode=SWI,
                    )
                # norms to SBUF (affine_then_add src1 must not be PSUM)
                nsqB = p_nsq2.tile([128, S_C], F32, tag="nsqB")
                for h in range(2):
                    nc.scalar.copy(nsqB[:, h * 512 : (h + 1) * 512], nps_s[h][:])
                bias_q = p_nsq2.tile([128, 1], F32, tag="biasq")
                nc.scalar.copy(bias_q[:], qn_ps[:])

                pending3 = (qs8, qsw, nsqB, bias_q)

            # drain the last rep's phases 3 and 4
            pending4 = phase3(*pending3)
            phase4(*pending4)

    nc.compile()
    return nc


def prep_inputs(x, sx, sy, W_feat, proj_weight):
    f8 = ml_dtypes.float8_e4m3
    x = np.asarray(x, np.float32)
    sx = np.asarray(sx, np.float32)
    sy = np.asarray(sy).astype(np.int64)
    W = np.asarray(W_feat, np.float32)
    P = np.asarray(proj_weight, np.float32)

    WP = (W @ P).astype(np.float32) * SCALE
    wp_h = np.ascontiguousarray(
        WP.reshape(KC, 128, PC, 128).transpose(1, 2, 0, 3)
    ).astype(f8).reshape(128, PC, KC2, 2, 128)
    wp_h = np.ascontiguousarray(
        wp_h[:, :, :, :, ::-1].transpose(0, 1, 2, 4, 3)
    ).reshape(128, PC, KC * 128)
    xt = np.ascontiguousarray(x.T.reshape(KC, 128, B).transpose(1, 0, 2)).astype(f8)
    sxt = np.ascontiguousarray(
        sx.T.reshape(KC, 128, 8 * S_C).transpose(1, 0, 2)
    ).astype(f8)

    in_maps = []
    for c in range(8):
        rxt = np.empty((128, KC, N), f8)
        rxt[:, :, :B] = xt
        rxt[:, :, B:] = sxt[:, :, c * S_C : (c + 1) * S_C]
        sy_c = sy[c * S_C : (c + 1) * S_C]
        oh = np.zeros((S_C, CPAD), np.float32)
        oh[np.arange(S_C), sy_c] = 1.0
        oh_h = np.ascontiguousarray(
            oh.reshape(SC, 128, CPAD).transpose(1, 0, 2)
        ).astype(f8)
        in_maps.append({"wp": wp_h, "rxt": rxt, "oh": oh_h})
    return in_maps


def combine_outputs(outs):
    total = np.zeros((B, CPAD), np.float64)
    for o in outs:
        total += o.astype(np.float64)
    Z = total.sum(axis=1)
    return np.log(total[:, :1000] / Z[:, None] + EPS).astype(np.float32)


_NC_CACHE = {}


def kernel(x, sx, sy, W_feat, proj_weight):
    in_maps = prep_inputs(x, sx, sy, W_feat, proj_weight)
    if "nc" not in _NC_CACHE:
        _NC_CACHE["nc"] = build_bass()
    nc = _NC_CACHE["nc"]
    last_err = None
    for _attempt in range(2):
        try:
            res = run_bass_kernel_spmd(nc, in_maps, list(range(8))).results
            return combine_outputs([res[c]["outp"] for c in range(8)])
        except Exception as e:
            last_err = e
            import time as _time

            _time.sleep(2.0)
    raise last_err

